# revision 1
# baseline (speedup 1.0000x reference)
"""Trainium2 Bass kernel for nn_Bert_69698729280007.

Data-parallel over batch: core b processes batch row b (2 chunks of 512
tokens through the 4-layer BERT encoder), then does its own offset-based
segment mean-pool.  No collectives.

Device-side layout: the residual stream is kept TRANSPOSED [D, tokens]
in fp32 (6 tiles of [128, 512] per chunk).  All GEMMs run in bf16 with
fp32 PSUM accumulation; LayerNorm statistics are computed with fp32r
ones-matmuls over the fp32 residual.  Softmax is computed in the
transposed orientation (keys on partitions): exp without max-subtraction
(scores are bounded ~2 for this model family), denominators via
ones-matmuls, normalization folded onto ctx^T.  The segment mean-pool is
one mask-matmul: G[t, w] = (st_w <= t < ed_w) built on device, then
out[w, d] = sum_t G[t,w] * h[t, d] scaled per-partition by valid/count.
"""

import os
import sys
from contextlib import ExitStack

import numpy as np
import ml_dtypes

for _p in ("/opt/trn_rl_repo", "/root/.axon_site/_ro/trn_rl_repo"):
    if os.path.isdir(_p) and _p not in sys.path:
        sys.path.append(_p)

import concourse.bass as bass
import concourse.tile as tile
from concourse import bacc, mybir
from concourse.bass_utils import run_bass_kernel_spmd
from concourse.masks import make_identity

AF = mybir.ActivationFunctionType
ALU = mybir.AluOpType
F32 = mybir.dt.float32
F32R = mybir.dt.float32r
BF16 = mybir.dt.bfloat16
I32 = mybir.dt.int32

B, S, W = 8, 1024, 512
D, H, F, L, V = 768, 12, 3072, 4, 28996
CH = 512
EPS = 1e-12
P = 128
DT = D // P          # 6 d-tiles
FT = F // P          # 24 f-tiles
NH = H // 2          # 6 head pairs
KT = CH // P         # 4 key tiles per chunk
DH = D // H          # 64

# columns in the per-layer "smalls" tensor [L, 128, 78]
_COLS = dict(bq=(0, 6), bk=(6, 6), bv=(12, 6), bo=(18, 6), b1f=(24, 24),
             b2f=(48, 6), g1=(54, 6), b1=(60, 6), g2=(66, 6), b2=(72, 6))

N_CORES = 8


def _col(sm, name, i):
    off, _n = _COLS[name]
    return sm[:, off + i:off + i + 1]


def build_kernel(ctx: ExitStack, tc: tile.TileContext, io: dict):
    nc = tc.nc

    consts = ctx.enter_context(tc.tile_pool(name="consts", bufs=1))
    big = ctx.enter_context(tc.tile_pool(name="big", bufs=1))
    psum = ctx.enter_context(tc.tile_pool(name="psum", bufs=1, space="PSUM"))

    # ---- constants ----
    ident_bf = consts.tile([P, P], BF16, tag="idbf")
    make_identity(nc, ident_bf)
    ident_f32 = consts.tile([P, P], F32, tag="idf32")
    make_identity(nc, ident_f32)
    ones64 = consts.tile([P, 64], BF16, tag="ones64")
    nc.vector.memset(ones64, 1.0)
    ones_b = consts.tile([P, P], BF16, tag="onesb")
    nc.vector.memset(ones_b, 1.0)

    # attention mask bias: [128, 8] (t-tile per column), -(1-m)*1e4
    mask_sb = consts.tile([P, 8], F32, tag="masksb")
    nc.sync.dma_start(out=mask_sb, in_=io["mask128"])
    mb = consts.tile([P, 8], F32, tag="mb")
    nc.vector.tensor_scalar(mb, mask_sb, 10000.0, -10000.0,
                            op0=ALU.mult, op1=ALU.add)

    # embedding gamma/beta broadcast along partitions [128, 768]
    gb_emb = consts.tile([P, 2, D], F32, tag="gbemb")
    nc.sync.dma_start(out=gb_emb, in_=io["emb_gb"][0:1, :, :].to_broadcast([P, 2, D]))

    # pos+type embedding, natural layout [128, 4, 768] (partition-first)
    pt_sb = consts.tile([P, 4, D], F32, tag="ptsb")
    nc.sync.dma_start(out=pt_sb, in_=io["pos_type"].rearrange("(t p) d -> p t d", p=P))

    # final-h natural-layout tiles (bf16), persist until pooling
    h_nat = [big.tile([P, D], BF16, tag="hnat", bufs=8, name=f"hnat{t}")
             for t in range(8)]

    work_ctx = ExitStack()
    work = work_ctx.enter_context(tc.tile_pool(name="work", bufs=1))

    def ln_txp(xpre, sm, gname, bname, tag):
        """LayerNorm over partition dim (D) of transposed tiles.

        xpre: list of 6 fp32 [128, 512] tiles (pre-LN).  Returns
        (x32, xb): fp32 and bf16 post-LN tile lists."""
        ps1 = psum.tile([P, CH], F32, tag="mm", bufs=2, name="lnps1")
        ps2 = psum.tile([P, CH], F32, tag="mm", bufs=2, name="lnps2")
        for k in range(DT):
            xb16 = work.tile([P, CH], BF16, tag="sqb", bufs=2, name="lnxb16")
            nc.vector.tensor_copy(xb16, xpre[k])
            nc.tensor.matmul(ps1, ones_b, xb16,
                             start=(k == 0), stop=(k == DT - 1))
            sq = work.tile([P, CH], BF16, tag="sq", bufs=2, name="lnsq")
            nc.vector.tensor_mul(sq, xb16, xb16)
            nc.tensor.matmul(ps2, ones_b, sq,
                             start=(k == 0), stop=(k == DT - 1))
        mean = work.tile([P, CH], F32, tag="stat", bufs=3, name="lnmean")
        nc.scalar.activation(mean, ps1, AF.Copy, scale=1.0 / D)
        m2 = work.tile([P, CH], F32, tag="stat", bufs=3, name="lnm2")
        nc.vector.tensor_mul(m2, mean, mean)
        istd = work.tile([P, CH], F32, tag="stat", bufs=3, name="lnistd")
        nc.vector.scalar_tensor_tensor(istd, ps2, 1.0 / D, m2,
                                       op0=ALU.mult, op1=ALU.subtract)
        nc.vector.tensor_scalar_add(istd, istd, EPS)
        nc.scalar.activation(istd, istd, AF.Sqrt)
        nc.vector.reciprocal(istd, istd)
        x32, xb = [], []
        for k in range(DT):
            xo = work.tile([P, CH], F32, tag="resid", bufs=12, name="lnx32")
            nc.vector.tensor_sub(xo, xpre[k], mean)
            nc.vector.tensor_mul(xo, xo, istd)
            nc.vector.tensor_scalar(xo, xo, _col(sm, gname, k),
                                    _col(sm, bname, k), op0=ALU.mult, op1=ALU.add)
            xc = work.tile([P, CH], BF16, tag="xb", bufs=12, name="lnxb")
            nc.vector.tensor_copy(xc, xo)
            x32.append(xo)
            xb.append(xc)
        return x32, xb

    for c in range(2):
        # ================= embedding (chunk c) =================
        X32 = [work.tile([P, CH], F32, tag="resid", bufs=12, name=f"embx32_{k}")
               for k in range(DT)]
        for tt in range(KT):
            ids_sb = work.tile([P, 1], I32, tag="ids", bufs=2, name="idssb")
            nc.sync.dma_start(out=ids_sb, in_=io["ids"][c * 4 + tt])
            eg = work.tile([P, D], F32, tag="embg", bufs=2, name="embg")
            nc.gpsimd.indirect_dma_start(
                out=eg, out_offset=None, in_=io["word_emb"][:],
                in_offset=bass.IndirectOffsetOnAxis(ap=ids_sb[:, :1], axis=0))
            nc.vector.tensor_add(eg, eg, pt_sb[:, tt, :])
            # natural-layout LN over free dim (768 = 3 x 256 bn_stats groups)
            stats = work.tile([P, 3, 6], F32, tag="bnst", bufs=2, name="bnst")
            egr = eg.rearrange("p (s q) -> p s q", s=3)
            for s in range(3):
                nc.vector.bn_stats(out=stats[:, s, :], in_=egr[:, s, :])
            mv = work.tile([P, 2], F32, tag="bnmv", bufs=2, name="bnmv")
            nc.vector.bn_aggr(out=mv, in_=stats)
            istd0 = work.tile([P, 1], F32, tag="bnis", bufs=2, name="bnis")
            nc.vector.tensor_scalar_add(istd0, mv[:, 1:2], EPS)
            nc.scalar.activation(istd0, istd0, AF.Sqrt)
            nc.vector.reciprocal(istd0, istd0)
            nc.vector.tensor_scalar(eg, eg, mv[:, 0:1], istd0,
                                    op0=ALU.subtract, op1=ALU.mult)
            nc.vector.tensor_mul(eg, eg, gb_emb[:, 0, :])
            nc.vector.tensor_add(eg, eg, gb_emb[:, 1, :])
            # transpose this token-tile into X^T
            for k in range(DT):
                pt = psum.tile([P, P], F32, tag="mm", bufs=2, name="embtp")
                nc.tensor.transpose(pt, eg[:, k * P:(k + 1) * P], ident_f32)
                nc.vector.tensor_copy(X32[k][:, tt * P:(tt + 1) * P], pt)
        Xb = []
        for k in range(DT):
            xc = work.tile([P, CH], BF16, tag="xb", bufs=12, name="embxb")
            nc.vector.tensor_copy(xc, X32[k])
            Xb.append(xc)

        # ================= encoder layers =================
        for l in range(L):
            sm = work.tile([P, 78], F32, tag="smalls", bufs=2, name="smalls")
            nc.sync.dma_start(out=sm, in_=io["smalls"][l])

            # ---- Q^T, K^T (transposed out) ----
            QT, KTt = [], []
            for which, wt, bn, dst in (("q", io["Wq"], "bq", QT),
                                       ("k", io["Wk"], "bk", KTt)):
                w_sb = work.tile([P, DT, D], BF16, tag="wmat", bufs=2,
                                 name=f"w{which}sb")
                nc.sync.dma_start(
                    out=w_sb, in_=wt[l].rearrange("(k p) n -> p k n", p=P))
                for m in range(DT):
                    ps = psum.tile([P, CH], F32, tag="mm", bufs=2, name="qkps")
                    for k in range(DT):
                        nc.tensor.matmul(
                            ps, w_sb[:, k, m * P:(m + 1) * P], Xb[k],
                            start=(k == 0), stop=(k == DT - 1))
                    o = work.tile([P, CH], BF16, tag=which, bufs=6, name=f"{which}t")
                    nc.vector.tensor_scalar_add(o, ps, _col(sm, bn, m))
                    dst.append(o)

            # ---- V (natural out, no bias: bv folded into ctx) ----
            wv_sb = work.tile([P, DT, D], BF16, tag="wmat", bufs=2, name="wvsb")
            nc.sync.dma_start(
                out=wv_sb, in_=io["Wv"][l].rearrange("(k p) n -> p k n", p=P))
            Vn = []
            for mt in range(KT):
                o = work.tile([P, D], BF16, tag="v", bufs=4, name="vnat")
                for nn in range(2):
                    ps = psum.tile([P, 384], F32, tag="mm", bufs=2, name="vps")
                    for k in range(DT):
                        nc.tensor.matmul(
                            ps, Xb[k][:, mt * P:(mt + 1) * P],
                            wv_sb[:, k, nn * 384:(nn + 1) * 384],
                            start=(k == 0), stop=(k == DT - 1))
                    nc.scalar.activation(o[:, nn * 384:(nn + 1) * 384], ps, AF.Copy)
                Vn.append(o)

            # ---- attention, one head at a time ----
            ctxT = []
            for p in range(NH):
                cx = work.tile([P, CH], BF16, tag="ctx", bufs=6, name="ctxt")
                for hh in range(2):
                    lo, hi = hh * 64, (hh + 1) * 64
                    Et = []
                    for jk in range(KT):
                        ps = psum.tile([P, CH], F32, tag="sc", bufs=2, name="scps")
                        nc.tensor.matmul(
                            ps, KTt[p][lo:hi, jk * P:(jk + 1) * P], QT[p][lo:hi, :],
                            start=True, stop=True)
                        e = work.tile([P, CH], BF16, tag="e", bufs=6, name="etile")
                        nc.scalar.activation(e, ps, AF.Exp, scale=0.125,
                                             bias=mb[:, c * 4 + jk: c * 4 + jk + 1])
                        Et.append(e)
                    psd = psum.tile([64, CH], F32, tag="dn", bufs=2, name="dnps")
                    for jk in range(KT):
                        nc.tensor.matmul(psd, ones64, Et[jk],
                                         start=(jk == 0), stop=(jk == KT - 1))
                    rec = work.tile([64, CH], F32, tag="rd", bufs=2, name="recd")
                    nc.vector.reciprocal(rec, psd)
                    psc = psum.tile([64, CH], F32, tag="cx", bufs=2, name="cxps")
                    h = 2 * p + hh
                    for jk in range(KT):
                        nc.tensor.matmul(psc, Vn[jk][:, h * DH:(h + 1) * DH],
                                         Et[jk],
                                         start=(jk == 0), stop=(jk == KT - 1))
                    nc.vector.tensor_mul(cx[lo:hi, :], psc, rec)
                    nc.vector.tensor_scalar_add(cx[lo:hi, :], cx[lo:hi, :],
                                                _col(sm, "bv", p)[lo:hi, :])
                ctxT.append(cx)

            # ---- O projection + residual ----
            wo_sb = work.tile([P, DT, D], BF16, tag="wmat", bufs=2, name="wosb")
            nc.sync.dma_start(
                out=wo_sb, in_=io["Wo"][l].rearrange("(k p) n -> p k n", p=P))
            X1pre = []
            for m in range(DT):
                ps = psum.tile([P, CH], F32, tag="mm", bufs=2, name="ops")
                for k in range(DT):
                    nc.tensor.matmul(
                        ps, wo_sb[:, k, m * P:(m + 1) * P], ctxT[k],
                        start=(k == 0), stop=(k == DT - 1))
                xp = work.tile([P, CH], F32, tag="pre", bufs=6, name="x1pre")
                nc.vector.scalar_tensor_tensor(xp, ps, _col(sm, "bo", m), X32[m],
                                               op0=ALU.add, op1=ALU.add)
                X1pre.append(xp)
            X32, Xb = ln_txp(X1pre, sm, "g1", "b1", "ln1")

            # ---- FFN ----
            H1 = []
            for mg in range(DT):
                w1_sb = work.tile([P, DT, CH], BF16, tag="w1", bufs=2, name="w1sb")
                nc.sync.dma_start(
                    out=w1_sb,
                    in_=io["W1"][l].rearrange("(k p) n -> p k n", p=P)
                    [:, :, mg * CH:(mg + 1) * CH])
                for mm in range(4):
                    ps = psum.tile([P, CH], F32, tag="mm", bufs=2, name="f1ps")
                    for k in range(DT):
                        nc.tensor.matmul(
                            ps, w1_sb[:, k, mm * P:(mm + 1) * P],
                            Xb[k], start=(k == 0), stop=(k == DT - 1))
                    hh1 = work.tile([P, CH], BF16, tag="h1", bufs=24, name="h1t")
                    nc.scalar.activation(hh1, ps, AF.Gelu,
                                         bias=_col(sm, "b1f", mg * 4 + mm))
                    H1.append(hh1)
            X2pre = []
            for m in range(DT):
                w2_sb = work.tile([P, FT, P], BF16, tag="w2", bufs=2, name="w2sb")
                nc.sync.dma_start(
                    out=w2_sb,
                    in_=io["W2"][l].rearrange("(k p) n -> p k n", p=P)
                    [:, :, m * P:(m + 1) * P])
                ps = psum.tile([P, CH], F32, tag="mm", bufs=2, name="f2ps")
                for k in range(FT):
                    nc.tensor.matmul(ps, w2_sb[:, k, :], H1[k],
                                     start=(k == 0), stop=(k == FT - 1))
                xp = work.tile([P, CH], F32, tag="pre", bufs=6, name="x2pre")
                nc.vector.scalar_tensor_tensor(xp, ps, _col(sm, "b2f", m), X32[m],
                                               op0=ALU.add, op1=ALU.add)
                X2pre.append(xp)
            X32, Xb = ln_txp(X2pre, sm, "g2", "b2", "ln2")

        # ---- transpose final h back to natural layout (bf16) ----
        for k in range(DT):
            for tt in range(KT):
                pt = psum.tile([P, P], BF16, tag="mm", bufs=2, name="fintp")
                nc.tensor.transpose(pt, Xb[k][:, tt * P:(tt + 1) * P], ident_bf)
                nc.vector.tensor_copy(h_nat[c * 4 + tt][:, k * P:(k + 1) * P], pt)

    # ================= segment mean-pool =================
    work_ctx.close()
    work = ctx.enter_context(tc.tile_pool(name="poolph", bufs=1))
    stb = work.tile([P, W], F32, tag="stb", bufs=1, name="stb")
    nc.sync.dma_start(out=stb, in_=io["st_row"][0:1, :].to_broadcast([P, W]))
    edb = work.tile([P, W], F32, tag="edb", bufs=1, name="edb")
    nc.sync.dma_start(out=edb, in_=io["ed_row"][0:1, :].to_broadcast([P, W]))

    Gt = []
    for t in range(8):
        it = work.tile([P, 1], F32, tag="iota", bufs=2, name="iotat")
        nc.sync.dma_start(out=it, in_=io["iota8"][t])
        g = work.tile([P, W], BF16, tag="g", bufs=8, name="gtile")
        nc.vector.tensor_scalar(g, stb, it, None, op0=ALU.is_le)
        g2 = work.tile([P, W], BF16, tag="g2", bufs=2, name="g2tile")
        nc.vector.tensor_scalar(g2, edb, it, None, op0=ALU.is_gt)
        nc.vector.tensor_mul(g, g, g2)
        Gt.append(g)

    # rmask[w] = (x_mask != 0 && st < ed) / max(ed - st, 1), laid out [128, 4]
    stp = work.tile([P, 4], F32, tag="stp", bufs=1, name="stp")
    nc.sync.dma_start(out=stp, in_=io["stp"])
    edp = work.tile([P, 4], F32, tag="edp", bufs=1, name="edp")
    nc.sync.dma_start(out=edp, in_=io["edp"])
    xmp = work.tile([P, 4], F32, tag="xmp", bufs=1, name="xmp")
    nc.sync.dma_start(out=xmp, in_=io["xmp"])
    rmask = work.tile([P, 4], F32, tag="rmask", bufs=1, name="rmask")
    nc.vector.tensor_sub(rmask, edp, stp)
    nc.vector.tensor_scalar_max(rmask, rmask, 1.0)
    nc.vector.reciprocal(rmask, rmask)
    t1 = work.tile([P, 4], F32, tag="pt1", bufs=1, name="pt1")
    nc.vector.tensor_scalar(t1, xmp, 0.0, None, op0=ALU.not_equal)
    nc.vector.tensor_mul(rmask, rmask, t1)
    nc.vector.tensor_tensor(t1, stp, edp, op=ALU.is_lt)
    nc.vector.tensor_mul(rmask, rmask, t1)

    for w in range(4):
        for dn in range(2):
            ps = psum.tile([P, 384], F32, tag="mm", bufs=2, name="poolps")
            for t in range(8):
                nc.tensor.matmul(ps, Gt[t][:, w * P:(w + 1) * P],
                                 h_nat[t][:, dn * 384:(dn + 1) * 384],
                                 start=(t == 0), stop=(t == 7))
            o = work.tile([P, 384], F32, tag="poolo", bufs=2, name="poolo")
            nc.scalar.activation(o, ps, AF.Copy, scale=rmask[:, w:w + 1])
            nc.sync.dma_start(
                out=io["out"][w * P:(w + 1) * P, dn * 384:(dn + 1) * 384], in_=o)


def build_program():
    nc = bacc.Bacc("TRN2", target_bir_lowering=False, debug=False,
                   num_devices=N_CORES)
    io = {}

    def inp(name, shape, dt):
        io[name] = nc.dram_tensor(name, list(shape), dt, kind="ExternalInput").ap()

    inp("ids", (8, P, 1), I32)
    inp("mask128", (P, 8), F32)
    inp("st_row", (1, W), F32)
    inp("ed_row", (1, W), F32)
    inp("stp", (P, 4), F32)
    inp("edp", (P, 4), F32)
    inp("xmp", (P, 4), F32)
    inp("iota8", (8, P, 1), F32)
    inp("word_emb", (V, D), F32)
    inp("pos_type", (CH, D), F32)
    inp("emb_gb", (1, 2, D), F32)
    inp("smalls", (L, P, 78), F32)
    inp("Wq", (L, D, D), BF16)
    inp("Wk", (L, D, D), BF16)
    inp("Wv", (L, D, D), BF16)
    inp("Wo", (L, D, D), BF16)
    inp("W1", (L, D, F), BF16)
    inp("W2", (L, F, D), BF16)
    io["out"] = nc.dram_tensor("out", [W, D], F32, kind="ExternalOutput").ap()

    with tile.TileContext(nc) as tc:
        with ExitStack() as ctx:
            build_kernel(ctx, tc, io)
    nc.compile()
    return nc


_NC_CACHE = None


def _get_program():
    global _NC_CACHE
    if _NC_CACHE is None:
        _NC_CACHE = build_program()
    return _NC_CACHE


def make_in_maps(inputs):
    """Host-side prep: shard per batch row, reshape/cast into device layouts."""
    bf = ml_dtypes.bfloat16
    x_bert = np.asarray(inputs["x_bert"])
    x_mask_tok = np.asarray(inputs["x_bert_mask"], dtype=np.float32)
    off = np.asarray(inputs["x_bert_offset"])
    xm = np.asarray(inputs["x_mask"])
    word_emb = np.ascontiguousarray(np.asarray(inputs["word_emb"], np.float32))
    pos_type = np.asarray(inputs["pos_emb"], np.float32) + \
        np.asarray(inputs["type_emb"], np.float32)[0][None, :]
    pos_type = np.ascontiguousarray(pos_type)
    emb_gb = np.stack([np.asarray(inputs["emb_g"], np.float32),
                       np.asarray(inputs["emb_b"], np.float32)])[None]
    emb_gb = np.ascontiguousarray(emb_gb)

    smalls = np.zeros((L, P, 78), np.float32)
    for nm, key in (("bq", "bq"), ("bk", "bk"), ("bv", "bv"), ("bo", "bo"),
                    ("b1f", "b1f"), ("b2f", "b2f"), ("g1", "ln1_g"),
                    ("b1", "ln1_b"), ("g2", "ln2_g"), ("b2", "ln2_b")):
        offc, n = _COLS[nm]
        arr = np.asarray(inputs[key], np.float32)  # [L, n*128]
        smalls[:, :, offc:offc + n] = arr.reshape(L, n, P).transpose(0, 2, 1)

    wts = {k: np.ascontiguousarray(np.asarray(inputs[k], np.float32).astype(bf))
           for k in ("Wq", "Wk", "Wv", "Wo", "W1", "W2")}
    iota8 = np.arange(S, dtype=np.float32).reshape(8, P, 1)

    in_maps = []
    for b in range(N_CORES):
        ids = np.ascontiguousarray(
            x_bert[b].astype(np.int32).reshape(8, P, 1))
        mask128 = np.ascontiguousarray(
            x_mask_tok[b].reshape(8, P).T.astype(np.float32))
        st = off[b, :, 0].astype(np.float32)
        ed = off[b, :, 1].astype(np.float32)
        m = {
            "ids": ids,
            "mask128": mask128,
            "st_row": st[None, :].copy(),
            "ed_row": ed[None, :].copy(),
            "stp": np.ascontiguousarray(st.reshape(4, P).T),
            "edp": np.ascontiguousarray(ed.reshape(4, P).T),
            "xmp": np.ascontiguousarray(
                xm[b].astype(np.float32).reshape(4, P).T),
            "iota8": iota8,
            "word_emb": word_emb,
            "pos_type": pos_type,
            "emb_gb": emb_gb,
            "smalls": smalls,
        }
        m.update(wts)
        in_maps.append(m)
    return in_maps


def kernel(**inputs):
    nc = _get_program()
    in_maps = make_in_maps(inputs)
    res = run_bass_kernel_spmd(nc, in_maps, list(range(N_CORES)))
    return np.stack([res.results[b]["out"] for b in range(N_CORES)])

